# revision 36
# baseline (speedup 1.0000x reference)
"""Trainium2 Bass kernel for nn_BetaVAEMark10Decoder.

Network (per sample): latent(4) -> Linear(256)+leaky -> reshape (1,8,32)
 -> convT(5,2)s(5,2) -> conv3x3 SAME +leaky   (5,16,16)
 -> convT(5,2)s(5,2) -> conv3x3 SAME +leaky   (25,32,8)
 -> convT(2,2)s(2,2) -> conv3x3 SAME +relu    (50,64,6)  -> NCHW out.

Each convT(k=s) + 3x3 pair composes into one exact linear map, block-banded
over rows: out row y reads 1-2 input rows through per-phase matrices.

Cost model facts this kernel is built around:
  * matmul cost = out_free_size x cycles_per_row (independent of K);
    f32r is 1 cyc/row only when N >= 256; fp16/bf16 are 1 cyc/row always.
  * DMA transfers serialize at ~360 GB/s -> write the output as fp16.
  * Activation cost = free_size x 0.83ns + ~0.4us fixed -> merge acts
    across PSUM banks.

Layers:
  L1-L3 run form 0 (feature-major, N=512 batch free) in f32r.
  L4 runs form 1 (batch-major out) in fp16 with support-split x-groups:
    out cols x in [0,33) only need input cols j0-15 (one 128-part chunk)
    and x in [31,64) only need j16-31 (the j15/j16-only rows for x31/32
    are naturally embedded in each chunk's weight slice; the two matmuls
    overlap-accumulate on cols 31-32).  This halves L4 PE time vs. the
    2-pass K-chunk split.

Sharding: pure data parallel, batch 4096 -> 8 cores x 512.
"""

import sys

import numpy as np

sys.path.insert(0, "/opt/trn_rl_repo")

import concourse.bass as bass  # noqa: E402
import concourse.bacc as bacc  # noqa: E402
import concourse.mybir as mybir  # noqa: E402
from concourse import tile  # noqa: E402
from concourse.bass_utils import run_bass_kernel_spmd  # noqa: E402

N_CORES = 8
B = 4096
BL = B // N_CORES  # 512 per core
F32 = mybir.dt.float32
F32R = mybir.dt.float32r
F16 = mybir.dt.float16


# ---------------------------------------------------------------- host math
def _fused_matrices(Wup, Wc, sy, sx, Win, in_idx, out_idx, n_out_cols):
    """Compose convT(k=s=(sy,sx)) with 3x3 SAME conv into per-phase row
    matrices.  Returns {(p, delta): M} where out row y (p = y%sy, i = y//sy)
    accumulates  in_row[i+delta] @ M[(p, delta)]  over available deltas.
    x-edge clipping is baked into M; y-edge clipping == skipping absent rows.
    """
    Wup = np.asarray(Wup, np.float32)
    Wc = np.asarray(Wc, np.float32)
    Cin = Wup.shape[2]
    Wout = Win * sx
    mats = {}
    for p in range(sy):
        deltas = {0}
        if p == 0:
            deltas.add(-1)
        if p == sy - 1:
            deltas.add(1)
        for d in sorted(deltas):
            M = np.zeros((Win * Cin, n_out_cols), np.float32)
            y = sy + p  # representative interior row
            i_t = y // sy + d
            nz = False
            for dy in (-1, 0, 1):
                yp = y + dy
                if yp // sy != i_t:
                    continue
                py = yp % sy
                for x in range(Wout):
                    for dx in (-1, 0, 1):
                        xp = x + dx
                        if xp < 0 or xp >= Wout:
                            continue
                        j, qx = divmod(xp, sx)
                        # conv_transpose (transpose_kernel=False) applies the
                        # spatially mirrored kernel per phase
                        CC = Wup[sy - 1 - py, sx - 1 - qx] @ Wc[dy + 1, dx + 1]
                        M[np.ix_(in_idx(j), out_idx(x))] += CC
                        nz = True
            if nz:
                mats[(p, d)] = M
    return mats


def build_host_matrices(W_lin, W_up1, W_c1, W_up2, W_c2, W_up3, W_c3):
    # L2 input = h natural ordering: feat = c*8 + j   (c<32, j<8)
    r2 = _fused_matrices(
        W_up1, W_c1, 5, 2, 8,
        in_idx=lambda j: np.arange(32) * 8 + j,
        out_idx=lambda x: x * 16 + np.arange(16),
        n_out_cols=256,
    )
    # L3 input ordering: feat = j*16 + c ; output feat = x*8 + o
    r3 = _fused_matrices(
        W_up2, W_c2, 5, 2, 16,
        in_idx=lambda j: j * 16 + np.arange(16),
        out_idx=lambda x: x * 8 + np.arange(8),
        n_out_cols=256,
    )
    # L4 input ordering: feat = j*8 + c ; output col = o*64 + x  (x contig)
    r4 = _fused_matrices(
        W_up3, W_c3, 2, 2, 32,
        in_idx=lambda j: j * 8 + np.arange(8),
        out_idx=lambda x: x + 64 * np.arange(6),
        n_out_cols=384,
    )
    return np.asarray(W_lin, np.float32), r2, r3, r4


def _contribs(p, i, n_in_rows, mats, sy):
    out = []
    for d in (-1, 0, 1):
        if (p, d) in mats and 0 <= i + d < n_in_rows:
            out.append((i + d, mats[(p, d)]))
    return out


def numpy_forward(latent, W_lin, b_lin, r2, r3, r4):
    """Pure-numpy forward through the fused matrices (golden check)."""
    def leaky(x):
        return np.where(x > 0, x, 0.01 * x)

    h = leaky(latent.astype(np.float32) @ W_lin + b_lin)  # [B, 256]
    rows = h[:, None, :]  # [B, 1, 256]
    for (mats, sy, n_in) in ((r2, 5, 1), (r3, 5, 5)):
        nrows = n_in * sy
        out = np.zeros((h.shape[0], nrows, 256), np.float32)
        for y in range(nrows):
            i, p = divmod(y, sy)
            for (src, M) in _contribs(p, i, n_in, mats, sy):
                out[:, y] += rows[:, src] @ M
        rows = leaky(out)
    out = np.zeros((h.shape[0], 50, 384), np.float32)
    for y in range(50):
        i, p = divmod(y, 2)
        for (src, M) in _contribs(p, i, 25, r4, 2):
            out[:, y] += rows[:, src] @ M
    out = np.maximum(out, 0.0)
    # [B, 50, 6, 64] -> NCHW [B, 6, 50, 64]
    return out.reshape(-1, 50, 6, 64).transpose(0, 2, 1, 3)


# ---------------------------------------------------------------- bass build
_CACHED = {}

# L4 support-split column groups (out col = o*64 + x):
#   A0: x in [0, 31)  -> needs only j0-15  (input partitions   0:128)
#   A1: x in [33, 64) -> needs only j16-31 (input partitions 128:256)
#   mid: x in {31, 32} -> j15 (parts 96:128 of chunk0, rows zero-padded)
#                        + j16 (parts 0:32 of chunk1, rows zero-padded)
XA0 = list(range(0, 31))
XA1 = list(range(33, 64))
XMID = [31, 32]


def _mat_names(tag, mats):
    return {k: f"{tag}_{k[0]}_{'m' if k[1] < 0 else 'p'}{abs(k[1])}" for k in mats}


def build_nc(r2_keys, r3_keys, r4_keys, has_bias):
    nc = bacc.Bacc('TRN2', target_bir_lowering=False, debug=False,
                   num_devices=N_CORES)

    lw = nc.declare_dram_parameter("lw", [4, BL + 256], F32R, isOutput=False)
    blin = nc.declare_dram_parameter("bl", [128, 2], F32, isOutput=False)
    r2n = _mat_names("r2", r2_keys)
    r3n = _mat_names("r3", r3_keys)
    r4n = _mat_names("r4", r4_keys)
    r2n = {k: nm for k, nm in r2n.items() if k[1] == 0}
    n2, n3, n4 = len(r2n), len(r3n), len(r4n)
    r2d = {nm: nc.declare_dram_parameter(nm, [128, 2, 256], F16, isOutput=False)
           for nm in r2n.values()}
    r3da = nc.declare_dram_parameter("r3a", [128, 3, 2, 256], F16, isOutput=False)
    r3db = nc.declare_dram_parameter("r3b", [128, n3 - 3, 2, 256], F16,
                                     isOutput=False)
    # per r4 key: a0 (6x31), a1 (6x31), m0 (6x2), m1 (6x2) = 396 cols
    r4d = nc.declare_dram_parameter("r4all", [128, n4, 396], F16, isOutput=False)
    # out stored (b, y, o, x) fp16; host transposes to NCHW + upcasts
    out = nc.declare_dram_parameter("out", [BL, 50, 6, 64], F16, isOutput=True)

    LR = mybir.ActivationFunctionType.Lrelu
    RELU = mybir.ActivationFunctionType.Relu

    with tile.TileContext(nc) as tc:
        with (
            tc.tile_pool(name="wpool", bufs=1) as wp,
            tc.tile_pool(name="acts", bufs=1) as ap,
            tc.tile_pool(name="ps", bufs=4, space=bass.MemorySpace.PSUM) as pp,
            tc.tile_pool(name="tmp", bufs=2) as tp,
            tc.tile_pool(name="outp", bufs=6) as op,
        ):
            lw_t = wp.tile([4, BL + 256], F32R, tag="lw")
            nc.sync.dma_start(out=lw_t[:], in_=lw[:])
            lat_t = lw_t[:, 0:BL]
            w1_t = lw_t[:, BL:BL + 256]
            if has_bias:
                bl_t = wp.tile([128, 2], F32, tag="bl")
                nc.sync.dma_start(out=bl_t[:], in_=blin[:])

            r2_t = {}
            for k, nm in r2n.items():
                t = wp.tile([128, 2, 256], F16, tag=nm)
                nc.sync.dma_start(out=t[:], in_=r2d[nm][:])
                r2_t[k] = t
            r3ta = wp.tile([128, 3, 2, 256], F16, tag="r3a")
            nc.sync.dma_start(out=r3ta[:], in_=r3da[:])
            r3tb = wp.tile([128, n3 - 3, 2, 256], F16, tag="r3b")
            nc.sync.dma_start(out=r3tb[:], in_=r3db[:])
            r4a = wp.tile([128, n4, 396], F16, tag="r4all")
            nc.sync.dma_start(out=r4a[:], in_=r4d[:])
            # r3 order: earliest-needed keys first
            r3order = sorted(r3n, key=lambda k: (max(0, k[0] - k[1] - 1), k[0]))
            r3_t = {k: (r3ta[:, ki] if ki < 3 else r3tb[:, ki - 3])
                    for ki, k in enumerate(r3order)}
            r4_t = {k: (r4a[:, ki, 0:198], r4a[:, ki, 198:396])
                    for ki, k in enumerate(r4n)}

            # ---- L1: h[256, B] = leaky(W_lin.T @ lat + b)
            x1 = ap.tile([128, 2, BL], F16, tag="x1")
            for mc in range(2):
                ps1 = pp.tile([128, 2, 8, 64], F32, tag="ps")
                nc.tensor.matmul(
                    ps1[:, 0, :, :],
                    lw_t[:, BL + mc * 128:BL + (mc + 1) * 128],
                    lw_t[:, 0:BL],
                    start=True, stop=True,
                )
                if has_bias:
                    nc.scalar.activation(
                        x1[:, mc, :], ps1[:, 0, :, :], LR,
                        bias=bl_t[:, mc:mc + 1], alpha=0.01,
                    )
                else:
                    nc.scalar.activation(
                        x1[:, mc, :], ps1[:, 0, :, :], LR, alpha=0.01,
                    )

            # ---- L2: 256 -> 1280 (5 rows x 256).  x2 slot = 2*y + mc.
            # Per-row PSUM tile + per-row Act leaky (latency-critical: L3
            # consumes x2 almost immediately).
            x2 = ap.tile([128, 10, BL], F16, tag="x2")
            for y in range(5):
                ps = pp.tile([128, 2, 8, 64], F32, tag="ps")
                cs = _contribs(y, 0, 1, r2_t, 5)
                for mc in range(2):
                    n, tot = 0, len(cs) * 2
                    for (src, mt) in cs:
                        for kc in range(2):
                            nc.tensor.matmul(
                                ps[:, mc, :, :],
                                mt[:, kc, bass.ts(mc, 128)],
                                x1[:, kc, :],
                                start=(n == 0), stop=(n == tot - 1),
                            )
                            n += 1
                nc.scalar.activation(
                    x2[:, 2 * y:2 * y + 2, :], ps[:, :, :, :], LR, alpha=0.01,
                )

            # ---- L3: 1280 -> 6400 (25 rows x 256), fp16 out for L4.
            # x3 slot = 2*y + mc.  Per-row tiles; leaky on Act for 2/3 of
            # rows, DVE (2-instr mul+max) for every 3rd: keeps Act under the
            # PE rate.  DVE's extra latency is fine (L4 consumes much later).
            x3 = ap.tile([128, 50, BL], F16, tag="x3")

            def emit_l3_row(y):
                ps = pp.tile([128, 2, 8, 64], F32, tag="ps")
                i, p = divmod(y, 5)
                cs = _contribs(p, i, 5, r3_t, 5)
                for mc in range(2):
                    n, tot = 0, len(cs) * 2
                    for (src, mt) in cs:
                        for kc in range(2):
                            nc.tensor.matmul(
                                ps[:, mc, :, :],
                                mt[:, kc, bass.ts(mc, 128)],
                                x2[:, 2 * src + kc, :],
                                start=(n == 0), stop=(n == tot - 1),
                            )
                            n += 1
                if y not in (3, 8, 13, 18, 23):
                    nc.scalar.activation(
                        x3[:, 2 * y:2 * y + 2, :], ps[:, :, :, :], LR,
                        alpha=0.01,
                    )
                else:
                    tmp = tp.tile([128, 2, 8, 64], F32, tag="tmp")
                    nc.vector.tensor_scalar_mul(tmp[:], ps[:], 0.01)
                    nc.vector.scalar_tensor_tensor(
                        x3[:, 2 * y:2 * y + 2, :], ps[:], 1.0, tmp[:],
                        op0=mybir.AluOpType.mult, op1=mybir.AluOpType.max,
                    )

            # ---- L4 (form 1, fp16): 6400 -> 19200, batch-major, relu, DMA.
            # 2-row PSUM tiles (bufs=4 ring -> no PE stalls), one relu act
            # per tile alternating Act/DVE, one DMA per 4-row ob group.
            # bb0's groups are interleaved into the L3 row loop (deps allow
            # group g once x3 rows <= (g+3)//2+1 exist) to spread act + DMA
            # load across the whole timeline.
            actst = {"n": 0}

            def emit_l4_group(bb, g0):
                ys = list(range(g0, min(g0 + 4, 50)))
                ob = op.tile([128, 4, 6, 64], F16, tag="ob")
                finalg = bb == BL // 128 - 1 and g0 == 48
                step = 1 if finalg else 2
                for h0 in range(0, len(ys), step):
                    sub = ys[h0:h0 + step]
                    pc = pp.tile([128, 2, 8, 64], F32, tag="ps")
                    for yy, y in enumerate(sub):
                        i, p = divmod(y, 2)
                        cs = _contribs(p, i, 25, r4_t, 2)
                        nmm, tot = 0, len(cs) * 2
                        for (src, (wa0, wa1)) in cs:
                            la = x3[:, 2 * src, bass.ts(bb, 128)]
                            lb = x3[:, 2 * src + 1, bass.ts(bb, 128)]
                            nc.tensor.matmul(
                                pc[:, yy, 0:6, 0:33], la, wa0,
                                start=(nmm == 0), stop=(nmm == tot - 2),
                            )
                            nc.tensor.matmul(
                                pc[:, yy, 0:6, 31:64], lb, wa1,
                                start=False, stop=(nmm == tot - 2),
                            )
                            nmm += 2
                    nsub = len(sub)
                    if actst["n"] % 2 == 0:
                        nc.scalar.activation(
                            ob[:, h0:h0 + nsub, :, :],
                            pc[:, 0:nsub, 0:6, :], RELU)
                    else:
                        nc.vector.tensor_scalar_max(
                            ob[:, h0:h0 + nsub, :, :],
                            pc[:, 0:nsub, 0:6, :], 0.0)
                    actst["n"] += 1
                if finalg:
                    for q in range(len(ys)):
                        nc.sync.dma_start(
                            out=out[bass.ts(bb, 128), g0 + q:g0 + q + 1, :, :],
                            in_=ob[:, q:q + 1, :, :],
                        )
                elif bb == BL // 128 - 1 and g0 + 12 >= 50:
                    for q in range(0, len(ys), 2):
                        qe = min(q + 2, len(ys))
                        nc.sync.dma_start(
                            out=out[bass.ts(bb, 128), g0 + q:g0 + qe, :, :],
                            in_=ob[:, q:qe, :, :],
                        )
                else:
                    nc.sync.dma_start(
                        out=out[bass.ts(bb, 128), g0:g0 + len(ys), :, :],
                        in_=ob[:, 0:len(ys), :, :],
                    )

            for y in range(25):
                emit_l3_row(y)
            for bb in range(BL // 128):
                for g0 in range(0, 50, 4):
                    emit_l4_group(bb, g0)
    nc.compile()
    return nc


# ---------------------------------------------------------------- entry
def kernel(**inputs):
    latent = np.asarray(inputs["latent_vector"], np.float32)
    W_lin, r2, r3, r4 = build_host_matrices(
        inputs["W_lin"], inputs["W_up1"], inputs["W_c1"],
        inputs["W_up2"], inputs["W_c2"], inputs["W_up3"], inputs["W_c3"],
    )
    b_lin = np.asarray(inputs["b_lin"], np.float32)

    has_bias = bool(np.any(b_lin != 0.0))
    if "nc" not in _CACHED:
        _CACHED["nc"] = build_nc(r2, r3, r4, has_bias)
    nc = _CACHED["nc"]

    def pack(m):  # [256, N] -> [128, 2, N]
        return np.ascontiguousarray(
            m.reshape(2, 128, m.shape[1]).transpose(1, 0, 2))

    base = {"bl": np.ascontiguousarray(b_lin.reshape(2, 128).T)}
    for k, nm in _mat_names("r2", r2).items():
        if k[1] == 0:
            base[nm] = pack(r2[k]).astype(np.float16)
    r3keys = sorted(_mat_names("r3", r3),
                    key=lambda k: (max(0, k[0] - k[1] - 1), k[0]))
    r3stk = np.stack([pack(r3[k]) for k in r3keys], axis=1).astype(np.float16)
    base["r3a"] = np.ascontiguousarray(r3stk[:, 0:3])
    base["r3b"] = np.ascontiguousarray(r3stk[:, 3:])
    # L4 support-split weight pieces; cols of the [256, 384] matrix are
    # o*64 + x.  a0: rows j0-15 for x in XA0; a1: rows j16-31 for x in XA1;
    # m0/m1: rows j12-15 / j16-19 (only j15/j16 nonzero) for x in XMID.
    r4rows = []
    for k in _mat_names("r4", r4):
        cview = r4[k].reshape(256, 6, 64)
        r4rows.append(np.concatenate([
            cview[0:128][:, :, 0:33].reshape(128, -1),
            cview[128:256][:, :, 31:64].reshape(128, -1),
        ], axis=1))
    base["r4all"] = np.ascontiguousarray(
        np.stack(r4rows, axis=1)).astype(np.float16)

    in_maps = []
    for c in range(N_CORES):
        sh = latent[c * BL:(c + 1) * BL]
        in_maps.append({**base,
                        "lw": np.ascontiguousarray(
                            np.concatenate([sh.T, W_lin], axis=1))})

    _CACHED["maps"] = in_maps
    res = run_bass_kernel_spmd(nc, in_maps, list(range(N_CORES)))
    outs = [np.transpose(r["out"].astype(np.float32), (0, 2, 1, 3))
            for r in res.results]
    return np.ascontiguousarray(np.concatenate(outs, axis=0))


if __name__ == "__main__":
    rng = np.random.default_rng(0)
    fake = {
        "latent_vector": rng.standard_normal((B, 4)).astype(np.float32),
        "W_lin": rng.standard_normal((4, 256)).astype(np.float32) * 0.5,
        "b_lin": np.zeros(256, np.float32),
        "W_up1": rng.standard_normal((5, 2, 32, 32)).astype(np.float32) * 0.1,
        "W_c1": rng.standard_normal((3, 3, 32, 16)).astype(np.float32) * 0.1,
        "W_up2": rng.standard_normal((5, 2, 16, 16)).astype(np.float32) * 0.1,
        "W_c2": rng.standard_normal((3, 3, 16, 8)).astype(np.float32) * 0.1,
        "W_up3": rng.standard_normal((2, 2, 8, 8)).astype(np.float32) * 0.1,
        "W_c3": rng.standard_normal((3, 3, 8, 6)).astype(np.float32) * 0.1,
    }
    o = kernel(**fake)
    print("kernel out", o.shape, o.dtype)


# revision 37
# speedup vs baseline: 1.0035x; 1.0035x over previous
"""Trainium2 Bass kernel for nn_BetaVAEMark10Decoder.

Network (per sample): latent(4) -> Linear(256)+leaky -> reshape (1,8,32)
 -> convT(5,2)s(5,2) -> conv3x3 SAME +leaky   (5,16,16)
 -> convT(5,2)s(5,2) -> conv3x3 SAME +leaky   (25,32,8)
 -> convT(2,2)s(2,2) -> conv3x3 SAME +relu    (50,64,6)  -> NCHW out.

Each convT(k=s) + 3x3 pair composes into one exact linear map, block-banded
over rows: out row y reads 1-2 input rows through per-phase matrices.

Cost model facts this kernel is built around:
  * matmul cost = out_free_size x cycles_per_row (independent of K);
    f32r is 1 cyc/row only when N >= 256; fp16/bf16 are 1 cyc/row always.
  * DMA transfers serialize at ~360 GB/s -> write the output as fp16.
  * Activation cost = free_size x 0.83ns + ~0.4us fixed -> merge acts
    across PSUM banks.

Layers:
  L1-L3 run form 0 (feature-major, N=512 batch free) in f32r.
  L4 runs form 1 (batch-major out) in fp16 with support-split x-groups:
    out cols x in [0,33) only need input cols j0-15 (one 128-part chunk)
    and x in [31,64) only need j16-31 (the j15/j16-only rows for x31/32
    are naturally embedded in each chunk's weight slice; the two matmuls
    overlap-accumulate on cols 31-32).  This halves L4 PE time vs. the
    2-pass K-chunk split.

Sharding: pure data parallel, batch 4096 -> 8 cores x 512.
"""

import sys

import numpy as np

sys.path.insert(0, "/opt/trn_rl_repo")

import concourse.bass as bass  # noqa: E402
import concourse.bacc as bacc  # noqa: E402
import concourse.mybir as mybir  # noqa: E402
from concourse import tile  # noqa: E402
from concourse.bass_utils import run_bass_kernel_spmd  # noqa: E402

N_CORES = 8
B = 4096
BL = B // N_CORES  # 512 per core
F32 = mybir.dt.float32
F32R = mybir.dt.float32r
F16 = mybir.dt.float16


# ---------------------------------------------------------------- host math
def _fused_matrices(Wup, Wc, sy, sx, Win, in_idx, out_idx, n_out_cols):
    """Compose convT(k=s=(sy,sx)) with 3x3 SAME conv into per-phase row
    matrices.  Returns {(p, delta): M} where out row y (p = y%sy, i = y//sy)
    accumulates  in_row[i+delta] @ M[(p, delta)]  over available deltas.
    x-edge clipping is baked into M; y-edge clipping == skipping absent rows.
    """
    Wup = np.asarray(Wup, np.float32)
    Wc = np.asarray(Wc, np.float32)
    Cin = Wup.shape[2]
    Wout = Win * sx
    mats = {}
    for p in range(sy):
        deltas = {0}
        if p == 0:
            deltas.add(-1)
        if p == sy - 1:
            deltas.add(1)
        for d in sorted(deltas):
            M = np.zeros((Win * Cin, n_out_cols), np.float32)
            y = sy + p  # representative interior row
            i_t = y // sy + d
            nz = False
            for dy in (-1, 0, 1):
                yp = y + dy
                if yp // sy != i_t:
                    continue
                py = yp % sy
                for x in range(Wout):
                    for dx in (-1, 0, 1):
                        xp = x + dx
                        if xp < 0 or xp >= Wout:
                            continue
                        j, qx = divmod(xp, sx)
                        # conv_transpose (transpose_kernel=False) applies the
                        # spatially mirrored kernel per phase
                        CC = Wup[sy - 1 - py, sx - 1 - qx] @ Wc[dy + 1, dx + 1]
                        M[np.ix_(in_idx(j), out_idx(x))] += CC
                        nz = True
            if nz:
                mats[(p, d)] = M
    return mats


def build_host_matrices(W_lin, W_up1, W_c1, W_up2, W_c2, W_up3, W_c3):
    # L2 input = h natural ordering: feat = c*8 + j   (c<32, j<8)
    r2 = _fused_matrices(
        W_up1, W_c1, 5, 2, 8,
        in_idx=lambda j: np.arange(32) * 8 + j,
        out_idx=lambda x: x * 16 + np.arange(16),
        n_out_cols=256,
    )
    # L3 input ordering: feat = j*16 + c ; output feat = x*8 + o
    r3 = _fused_matrices(
        W_up2, W_c2, 5, 2, 16,
        in_idx=lambda j: j * 16 + np.arange(16),
        out_idx=lambda x: x * 8 + np.arange(8),
        n_out_cols=256,
    )
    # L4 input ordering: feat = j*8 + c ; output col = o*64 + x  (x contig)
    r4 = _fused_matrices(
        W_up3, W_c3, 2, 2, 32,
        in_idx=lambda j: j * 8 + np.arange(8),
        out_idx=lambda x: x + 64 * np.arange(6),
        n_out_cols=384,
    )
    return np.asarray(W_lin, np.float32), r2, r3, r4


def _contribs(p, i, n_in_rows, mats, sy):
    out = []
    for d in (-1, 0, 1):
        if (p, d) in mats and 0 <= i + d < n_in_rows:
            out.append((i + d, mats[(p, d)]))
    return out


def numpy_forward(latent, W_lin, b_lin, r2, r3, r4):
    """Pure-numpy forward through the fused matrices (golden check)."""
    def leaky(x):
        return np.where(x > 0, x, 0.01 * x)

    h = leaky(latent.astype(np.float32) @ W_lin + b_lin)  # [B, 256]
    rows = h[:, None, :]  # [B, 1, 256]
    for (mats, sy, n_in) in ((r2, 5, 1), (r3, 5, 5)):
        nrows = n_in * sy
        out = np.zeros((h.shape[0], nrows, 256), np.float32)
        for y in range(nrows):
            i, p = divmod(y, sy)
            for (src, M) in _contribs(p, i, n_in, mats, sy):
                out[:, y] += rows[:, src] @ M
        rows = leaky(out)
    out = np.zeros((h.shape[0], 50, 384), np.float32)
    for y in range(50):
        i, p = divmod(y, 2)
        for (src, M) in _contribs(p, i, 25, r4, 2):
            out[:, y] += rows[:, src] @ M
    out = np.maximum(out, 0.0)
    # [B, 50, 6, 64] -> NCHW [B, 6, 50, 64]
    return out.reshape(-1, 50, 6, 64).transpose(0, 2, 1, 3)


# ---------------------------------------------------------------- bass build
_CACHED = {}

# L4 support-split column groups (out col = o*64 + x):
#   A0: x in [0, 31)  -> needs only j0-15  (input partitions   0:128)
#   A1: x in [33, 64) -> needs only j16-31 (input partitions 128:256)
#   mid: x in {31, 32} -> j15 (parts 96:128 of chunk0, rows zero-padded)
#                        + j16 (parts 0:32 of chunk1, rows zero-padded)
XA0 = list(range(0, 31))
XA1 = list(range(33, 64))
XMID = [31, 32]


def _mat_names(tag, mats):
    return {k: f"{tag}_{k[0]}_{'m' if k[1] < 0 else 'p'}{abs(k[1])}" for k in mats}


def build_nc(r2_keys, r3_keys, r4_keys, has_bias):
    nc = bacc.Bacc('TRN2', target_bir_lowering=False, debug=False,
                   num_devices=N_CORES)

    lw = nc.declare_dram_parameter("lw", [4, BL + 256], F32R, isOutput=False)
    blin = nc.declare_dram_parameter("bl", [128, 2], F32, isOutput=False)
    r2n = _mat_names("r2", r2_keys)
    r3n = _mat_names("r3", r3_keys)
    r4n = _mat_names("r4", r4_keys)
    r2n = {k: nm for k, nm in r2n.items() if k[1] == 0}
    n2, n3, n4 = len(r2n), len(r3n), len(r4n)
    r2d = {nm: nc.declare_dram_parameter(nm, [128, 2, 256], F16, isOutput=False)
           for nm in r2n.values()}
    r3da = nc.declare_dram_parameter("r3a", [128, 3, 2, 256], F16, isOutput=False)
    r3db = nc.declare_dram_parameter("r3b", [128, n3 - 3, 2, 256], F16,
                                     isOutput=False)
    # per r4 key: a0 (6x31), a1 (6x31), m0 (6x2), m1 (6x2) = 396 cols
    r4d = nc.declare_dram_parameter("r4all", [128, n4, 396], F16, isOutput=False)
    # out stored (b, y, o, x) fp16; host transposes to NCHW + upcasts
    out = nc.declare_dram_parameter("out", [BL, 50, 6, 64], F16, isOutput=True)

    LR = mybir.ActivationFunctionType.Lrelu
    RELU = mybir.ActivationFunctionType.Relu

    with tile.TileContext(nc) as tc:
        with (
            tc.tile_pool(name="wpool", bufs=1) as wp,
            tc.tile_pool(name="acts", bufs=1) as ap,
            tc.tile_pool(name="ps", bufs=4, space=bass.MemorySpace.PSUM) as pp,
            tc.tile_pool(name="tmp", bufs=2) as tp,
            tc.tile_pool(name="outp", bufs=6) as op,
        ):
            lw_t = wp.tile([4, BL + 256], F32R, tag="lw")
            nc.sync.dma_start(out=lw_t[:], in_=lw[:])
            lat_t = lw_t[:, 0:BL]
            w1_t = lw_t[:, BL:BL + 256]
            if has_bias:
                bl_t = wp.tile([128, 2], F32, tag="bl")
                nc.sync.dma_start(out=bl_t[:], in_=blin[:])

            r2_t = {}
            for k, nm in r2n.items():
                t = wp.tile([128, 2, 256], F16, tag=nm)
                nc.sync.dma_start(out=t[:], in_=r2d[nm][:])
                r2_t[k] = t
            r3ta = wp.tile([128, 3, 2, 256], F16, tag="r3a")
            nc.sync.dma_start(out=r3ta[:], in_=r3da[:])
            r3tb = wp.tile([128, n3 - 3, 2, 256], F16, tag="r3b")
            nc.sync.dma_start(out=r3tb[:], in_=r3db[:])
            r4a = wp.tile([128, n4, 396], F16, tag="r4all")
            nc.sync.dma_start(out=r4a[:], in_=r4d[:])
            # r3 order: earliest-needed keys first
            r3order = sorted(r3n, key=lambda k: (max(0, k[0] - k[1] - 1), k[0]))
            r3_t = {k: (r3ta[:, ki] if ki < 3 else r3tb[:, ki - 3])
                    for ki, k in enumerate(r3order)}
            r4_t = {k: (r4a[:, ki, 0:198], r4a[:, ki, 198:396])
                    for ki, k in enumerate(r4n)}

            # ---- L1: h[256, B] = leaky(W_lin.T @ lat + b)
            x1 = ap.tile([128, 2, BL], F16, tag="x1")
            for mc in range(2):
                ps1 = pp.tile([128, 2, 8, 64], F32, tag="ps")
                nc.tensor.matmul(
                    ps1[:, 0, :, :],
                    lw_t[:, BL + mc * 128:BL + (mc + 1) * 128],
                    lw_t[:, 0:BL],
                    start=True, stop=True,
                )
                if has_bias:
                    nc.scalar.activation(
                        x1[:, mc, :], ps1[:, 0, :, :], LR,
                        bias=bl_t[:, mc:mc + 1], alpha=0.01,
                    )
                else:
                    nc.scalar.activation(
                        x1[:, mc, :], ps1[:, 0, :, :], LR, alpha=0.01,
                    )

            # ---- L2: 256 -> 1280 (5 rows x 256).  x2 slot = 2*y + mc.
            # Per-row PSUM tile + per-row Act leaky (latency-critical: L3
            # consumes x2 almost immediately).
            x2 = ap.tile([128, 10, BL], F16, tag="x2")
            for y in range(5):
                ps = pp.tile([128, 2, 8, 64], F32, tag="ps")
                cs = _contribs(y, 0, 1, r2_t, 5)
                for mc in range(2):
                    n, tot = 0, len(cs) * 2
                    for (src, mt) in cs:
                        for kc in range(2):
                            nc.tensor.matmul(
                                ps[:, mc, :, :],
                                mt[:, kc, bass.ts(mc, 128)],
                                x1[:, kc, :],
                                start=(n == 0), stop=(n == tot - 1),
                            )
                            n += 1
                nc.scalar.activation(
                    x2[:, 2 * y:2 * y + 2, :], ps[:, :, :, :], LR, alpha=0.01,
                )

            # ---- L3: 1280 -> 6400 (25 rows x 256), fp16 out for L4.
            # x3 slot = 2*y + mc.  Per-row tiles; leaky on Act for 2/3 of
            # rows, DVE (2-instr mul+max) for every 3rd: keeps Act under the
            # PE rate.  DVE's extra latency is fine (L4 consumes much later).
            x3 = ap.tile([128, 50, BL], F16, tag="x3")

            def emit_l3_row(y):
                ps = pp.tile([128, 2, 8, 64], F32, tag="ps")
                i, p = divmod(y, 5)
                cs = _contribs(p, i, 5, r3_t, 5)
                for mc in range(2):
                    n, tot = 0, len(cs) * 2
                    for (src, mt) in cs:
                        for kc in range(2):
                            nc.tensor.matmul(
                                ps[:, mc, :, :],
                                mt[:, kc, bass.ts(mc, 128)],
                                x2[:, 2 * src + kc, :],
                                start=(n == 0), stop=(n == tot - 1),
                            )
                            n += 1
                if y not in (3, 8, 13, 18, 23):
                    nc.scalar.activation(
                        x3[:, 2 * y:2 * y + 2, :], ps[:, :, :, :], LR,
                        alpha=0.01,
                    )
                else:
                    tmp = tp.tile([128, 2, 8, 64], F32, tag="tmp")
                    nc.vector.tensor_scalar_mul(tmp[:], ps[:], 0.01)
                    nc.vector.scalar_tensor_tensor(
                        x3[:, 2 * y:2 * y + 2, :], ps[:], 1.0, tmp[:],
                        op0=mybir.AluOpType.mult, op1=mybir.AluOpType.max,
                    )

            # ---- L4 (form 1, fp16): 6400 -> 19200, batch-major, relu, DMA.
            # 2-row PSUM tiles (bufs=4 ring -> no PE stalls), one relu act
            # per tile alternating Act/DVE, one DMA per 4-row ob group.
            # bb0's groups are interleaved into the L3 row loop (deps allow
            # group g once x3 rows <= (g+3)//2+1 exist) to spread act + DMA
            # load across the whole timeline.
            actst = {"n": 0}

            def emit_l4_group(bb, g0):
                ys = list(range(g0, min(g0 + 4, 50)))
                ob = op.tile([128, 4, 6, 64], F16, tag="ob")
                for h0 in range(0, len(ys), 2):
                    sub = ys[h0:h0 + 2]
                    pc = pp.tile([128, 2, 8, 64], F32, tag="ps")
                    for yy, y in enumerate(sub):
                        i, p = divmod(y, 2)
                        cs = _contribs(p, i, 25, r4_t, 2)
                        nmm, tot = 0, len(cs) * 2
                        for (src, (wa0, wa1)) in cs:
                            la = x3[:, 2 * src, bass.ts(bb, 128)]
                            lb = x3[:, 2 * src + 1, bass.ts(bb, 128)]
                            nc.tensor.matmul(
                                pc[:, yy, 0:6, 0:33], la, wa0,
                                start=(nmm == 0), stop=(nmm == tot - 2),
                            )
                            nc.tensor.matmul(
                                pc[:, yy, 0:6, 31:64], lb, wa1,
                                start=False, stop=(nmm == tot - 2),
                            )
                            nmm += 2
                    nsub = len(sub)
                    if actst["n"] % 2 == 0:
                        nc.scalar.activation(
                            ob[:, h0:h0 + nsub, :, :],
                            pc[:, 0:nsub, 0:6, :], RELU)
                    else:
                        nc.vector.tensor_scalar_max(
                            ob[:, h0:h0 + nsub, :, :],
                            pc[:, 0:nsub, 0:6, :], 0.0)
                    actst["n"] += 1
                if bb == BL // 128 - 1 and g0 + 12 >= 50:
                    for q in range(0, len(ys), 2):
                        qe = min(q + 2, len(ys))
                        nc.sync.dma_start(
                            out=out[bass.ts(bb, 128), g0 + q:g0 + qe, :, :],
                            in_=ob[:, q:qe, :, :],
                        )
                else:
                    nc.sync.dma_start(
                        out=out[bass.ts(bb, 128), g0:g0 + len(ys), :, :],
                        in_=ob[:, 0:len(ys), :, :],
                    )

            for y in range(25):
                emit_l3_row(y)
            for bb in range(BL // 128):
                for g0 in range(0, 50, 4):
                    emit_l4_group(bb, g0)
    nc.compile()
    return nc


# ---------------------------------------------------------------- entry
def kernel(**inputs):
    latent = np.asarray(inputs["latent_vector"], np.float32)
    W_lin, r2, r3, r4 = build_host_matrices(
        inputs["W_lin"], inputs["W_up1"], inputs["W_c1"],
        inputs["W_up2"], inputs["W_c2"], inputs["W_up3"], inputs["W_c3"],
    )
    b_lin = np.asarray(inputs["b_lin"], np.float32)

    has_bias = bool(np.any(b_lin != 0.0))
    if "nc" not in _CACHED:
        _CACHED["nc"] = build_nc(r2, r3, r4, has_bias)
    nc = _CACHED["nc"]

    def pack(m):  # [256, N] -> [128, 2, N]
        return np.ascontiguousarray(
            m.reshape(2, 128, m.shape[1]).transpose(1, 0, 2))

    base = {"bl": np.ascontiguousarray(b_lin.reshape(2, 128).T)}
    for k, nm in _mat_names("r2", r2).items():
        if k[1] == 0:
            base[nm] = pack(r2[k]).astype(np.float16)
    r3keys = sorted(_mat_names("r3", r3),
                    key=lambda k: (max(0, k[0] - k[1] - 1), k[0]))
    r3stk = np.stack([pack(r3[k]) for k in r3keys], axis=1).astype(np.float16)
    base["r3a"] = np.ascontiguousarray(r3stk[:, 0:3])
    base["r3b"] = np.ascontiguousarray(r3stk[:, 3:])
    # L4 support-split weight pieces; cols of the [256, 384] matrix are
    # o*64 + x.  a0: rows j0-15 for x in XA0; a1: rows j16-31 for x in XA1;
    # m0/m1: rows j12-15 / j16-19 (only j15/j16 nonzero) for x in XMID.
    r4rows = []
    for k in _mat_names("r4", r4):
        cview = r4[k].reshape(256, 6, 64)
        r4rows.append(np.concatenate([
            cview[0:128][:, :, 0:33].reshape(128, -1),
            cview[128:256][:, :, 31:64].reshape(128, -1),
        ], axis=1))
    base["r4all"] = np.ascontiguousarray(
        np.stack(r4rows, axis=1)).astype(np.float16)

    in_maps = []
    for c in range(N_CORES):
        sh = latent[c * BL:(c + 1) * BL]
        in_maps.append({**base,
                        "lw": np.ascontiguousarray(
                            np.concatenate([sh.T, W_lin], axis=1))})

    _CACHED["maps"] = in_maps
    res = run_bass_kernel_spmd(nc, in_maps, list(range(N_CORES)))
    outs = [np.transpose(r["out"].astype(np.float32), (0, 2, 1, 3))
            for r in res.results]
    return np.ascontiguousarray(np.concatenate(outs, axis=0))


if __name__ == "__main__":
    rng = np.random.default_rng(0)
    fake = {
        "latent_vector": rng.standard_normal((B, 4)).astype(np.float32),
        "W_lin": rng.standard_normal((4, 256)).astype(np.float32) * 0.5,
        "b_lin": np.zeros(256, np.float32),
        "W_up1": rng.standard_normal((5, 2, 32, 32)).astype(np.float32) * 0.1,
        "W_c1": rng.standard_normal((3, 3, 32, 16)).astype(np.float32) * 0.1,
        "W_up2": rng.standard_normal((5, 2, 16, 16)).astype(np.float32) * 0.1,
        "W_c2": rng.standard_normal((3, 3, 16, 8)).astype(np.float32) * 0.1,
        "W_up3": rng.standard_normal((2, 2, 8, 8)).astype(np.float32) * 0.1,
        "W_c3": rng.standard_normal((3, 3, 8, 6)).astype(np.float32) * 0.1,
    }
    o = kernel(**fake)
    print("kernel out", o.shape, o.dtype)
